# revision 8
# baseline (speedup 1.0000x reference)
"""Trainium2 Bass kernel for nn_AMCValueNet (ragged prefix-attention value net).

Math (per core, band rows i in [40c, 40c+40)): with A = Wq.T @ Wk folded on
host (weights-only preprocessing), the masked prefix attention collapses to

  S[i,n]  = x_i @ A @ x_n.T + w[n]        (w[n] = x_n.(Wk.T bq) + bq.bk;
                                           the per-row bias x_i.(Wq.T bk)
                                           cancels in P/Lc and is dropped)
  E       = exp(S/sqrt(d))
  Lc[i,t] = sum_{k<=t} E[i,k]             (prefix scan)
  P[i,t]  = sum_{k<=t} E[i,k] z[k]        (prefix scan of E*z, z = v@w1)
  t1      = sum_{i,j} 1{i<j} (1/j) P[i,j-1] / Lc[i,j-1]
  out     = t1 + w2 . sum_i x_i + n*bc    (last two terms on host)

Device work: G0 = x_band @ A (4 matmuls), S = G0 @ x.T + w (5 matmuls),
exp, one stacked [80, 319] scan (Lc rows 0:40, P rows 40:80), and a fused
multiply/divide/accumulate epilogue.  Everything else (A, w, z, bmrj masks)
is tiny host-side preprocessing.

Sharding: 8 cores each own a contiguous band of 40 query rows; the host
sums the per-core [40, 2] partial accumulators.
"""

import os
import numpy as np
import ml_dtypes

import concourse.bacc as bacc
import concourse.mybir as mybir
from concourse import tile
from concourse.bass_utils import run_bass_kernel_spmd

N = 320
D = 512
NCORES = 8
B = N // NCORES          # 40 query rows per core
PT = 128                 # partition tile
ND = D // PT             # 4 chunks of the contraction dims
H0, H1 = 160, 320        # column-halves for the epilogue pipeline
SCALE = 1.0 / float(np.sqrt(np.float32(D)))

F32 = mybir.dt.float32
BF16 = mybir.dt.bfloat16
BF16_NP = ml_dtypes.bfloat16

LAST_RESULT = None  # BassKernelResults of the most recent run (for test.py)
_CACHED_NC = None


def _ensure_ntff_hook():
    """Install the antenv.axon_hooks NTFF-profile shim if the container's
    antenv stub lacks it (mirrors trn_boot._ntff_profile_via_ctypes)."""
    import contextlib
    import ctypes
    import sys
    import types

    try:
        from antenv.axon_hooks import get_axon_ntff_profile_hook  # noqa: F401
        return
    except ImportError:
        pass
    so_path = "/opt/axon/libaxon_pjrt.so"
    if not os.path.exists(so_path):
        return
    lib = ctypes.CDLL(so_path)
    if not hasattr(lib, "axon_start_nrt_profile"):
        return
    lib.axon_start_nrt_profile.argtypes = [
        ctypes.POINTER(ctypes.c_int64), ctypes.c_size_t]
    lib.axon_start_nrt_profile.restype = ctypes.c_int64
    lib.axon_stop_nrt_profile.argtypes = [ctypes.c_char_p]
    lib.axon_stop_nrt_profile.restype = ctypes.c_int64

    @contextlib.contextmanager
    def _hook(output_dir, device_ids):
        import jax
        jax.devices()
        if device_ids:
            ids = (ctypes.c_int64 * len(device_ids))(*device_ids)
            rc = lib.axon_start_nrt_profile(ids, len(device_ids))
        else:
            rc = lib.axon_start_nrt_profile(None, 0)
        if rc != 0:
            raise RuntimeError(f"axon_start_nrt_profile rc={rc}")
        try:
            yield
        finally:
            n = lib.axon_stop_nrt_profile(str(output_dir).encode())
            print(f"profile: {n} ntff file(s) -> {output_dir}", file=sys.stderr)

    mod = types.ModuleType("antenv.axon_hooks")
    mod.get_axon_ntff_profile_hook = lambda: _hook
    mod.set_axon_ntff_profile_hook = lambda h: None
    import antenv
    antenv.axon_hooks = mod
    sys.modules["antenv.axon_hooks"] = mod


def _build_nc():
    nc = bacc.Bacc("TRN2", target_bir_lowering=False, debug=False)

    a_d = nc.dram_tensor("a", [PT, ND * D], BF16, kind="ExternalInput")
    xt_d = nc.dram_tensor("xt", [PT, ND * N], BF16, kind="ExternalInput")
    xtb_d = nc.dram_tensor("xtb", [PT, ND * B], BF16, kind="ExternalInput")
    misc_d = nc.dram_tensor("misc", [104, 2 * N], BF16, kind="ExternalInput")
    out_d = nc.dram_tensor("out", [B, 2], F32, kind="ExternalOutput")

    with tile.TileContext(nc) as tc:
        with (
            tc.tile_pool(name="w", bufs=1) as wpool,
            tc.tile_pool(name="pg", bufs=4, space="PSUM") as pg,   # [128,40] x4
            tc.tile_pool(name="ps", bufs=1, space="PSUM") as psp,  # [40,320]
        ):
            a_sb = wpool.tile([PT, ND, D], BF16, tag="a")
            xt_sb = wpool.tile([PT, ND, N], BF16, tag="xt")
            xtb_sb = wpool.tile([PT, ND, B], BF16, tag="xtb")
            # rows 0:40 = z broadcast, row 0 cols N:2N = w, rows 64:104 = mask/j
            misc_sb = wpool.tile([104, 2 * N], BF16, tag="misc")
            ones1 = wpool.tile([1, B], BF16, tag="ones1")
            g0t_sb = wpool.tile([PT, ND, B], BF16, tag="g0t")
            # E lives at partitions 0:40, Ez at 64:104 (partition starts must
            # be multiples of 32); rows 40:64, 104:128 are dead.
            es_sb = wpool.tile([PT, N], BF16, tag="es")
            sc_sb = wpool.tile([PT, N - 1], F32, tag="sc")
            pm_sb = wpool.tile([B, N - 1], BF16, tag="pm")
            rec_sb = wpool.tile([B, N - 1], F32, tag="rec")
            junk_sb = wpool.tile([B, N - 1], F32, tag="junk")
            acc_sb = wpool.tile([B, 2], F32, tag="acc")

            # ---- input DMAs: A halves on the two HW DGEs first (FIFO
            # priority), xT behind them, small stuff on the SW DGE. ----
            nc.sync.dma_start(a_sb[:, 0:2, :], a_d[:, 0:2 * D])
            nc.scalar.dma_start(a_sb[:, 2:4, :], a_d[:, 2 * D:])
            nc.gpsimd.dma_start(xtb_sb[:], xtb_d[:, :])
            nc.sync.dma_start(xt_sb[:, 0:2, :], xt_d[:, 0:2 * N])
            nc.scalar.dma_start(xt_sb[:, 2:4, :], xt_d[:, 2 * N:])
            nc.gpsimd.dma_start(misc_sb[:], misc_d[:, :])
            nc.gpsimd.memset(ones1[:], 1.0)
            # clear the dead rows so stale SBUF NaNs can't reach the scan
            nc.gpsimd.memset(es_sb[:], 0.0)

            # ---- G0.T = A.T @ x_band.T  ([512, 40] in 4 psum chunks) ----
            for r in range(ND):
                ps = pg.tile([PT, B], F32, tag="pg", name=f"g0t{r}")
                for d in range(ND):
                    nc.tensor.matmul(
                        ps[:], a_sb[:, d, r * PT:(r + 1) * PT],
                        xtb_sb[:, d, :],
                        start=(d == 0), stop=(d == ND - 1),
                    )
                nc.scalar.copy(g0t_sb[:, r, :], ps[:])

            # ---- S = G0 @ x.T + w  ([40, 320] psum) ----
            s_ps = psp.tile([B, N], F32, tag="ps")
            for r in range(ND):
                nc.tensor.matmul(s_ps[:], g0t_sb[:, r, :], xt_sb[:, r, :],
                                 start=(r == 0), stop=False)
            nc.tensor.matmul(s_ps[:], ones1[0:1, :], misc_sb[0:1, N:2 * N],
                             start=False, stop=True)

            # ---- E = exp(S/sqrt(d)) into es[0:40]; Ez = E*z into es[40:80],
            # by column-halves so vector trails scalar. ----
            for c0, c1 in ((0, H0), (H0, H1)):
                nc.scalar.activation(es_sb[0:B, c0:c1], s_ps[:, c0:c1],
                                     mybir.ActivationFunctionType.Exp,
                                     scale=SCALE)
                nc.vector.tensor_mul(es_sb[64:64 + B, c0:c1],
                                     es_sb[0:B, c0:c1], misc_sb[0:B, c0:c1])

            # ---- stacked prefix scans: Lc rows 0:40, P rows 40:80 ----
            nc.vector.tensor_tensor_scan(
                out=sc_sb[:, 0:H0], data0=es_sb[:, 0:H0],
                data1=es_sb[:, 0:H0], initial=0.0,
                op0=mybir.AluOpType.add, op1=mybir.AluOpType.bypass,
            )
            nc.vector.tensor_tensor_scan(
                out=sc_sb[:, H0:N - 1], data0=es_sb[:, H0:N - 1],
                data1=es_sb[:, H0:N - 1], initial=sc_sb[:, H0 - 1:H0],
                op0=mybir.AluOpType.add, op1=mybir.AluOpType.bypass,
            )

            # ---- t1 rows: acc[:, ch] = sum_t P*mask/Lc over column chunk ----
            for ch, (c0, c1) in enumerate(((0, H0), (H0, N - 1))):
                with nc.allow_low_precision(reason="bf16 mask product"):
                    nc.vector.scalar_tensor_tensor(
                        out=pm_sb[:, c0:c1], in0=sc_sb[64:64 + B, c0:c1],
                        scalar=1.0, in1=misc_sb[64:64 + B, 1 + c0:1 + c1],
                        op0=mybir.AluOpType.mult, op1=mybir.AluOpType.mult,
                    )
                nc.vector.reciprocal_approx_fast(
                    out=rec_sb[:, c0:c1], in_=sc_sb[0:B, c0:c1])
                nc.vector.scalar_tensor_tensor(
                    out=junk_sb[:, c0:c1], in0=pm_sb[:, c0:c1],
                    scalar=1.0, in1=rec_sb[:, c0:c1],
                    op0=mybir.AluOpType.mult, op1=mybir.AluOpType.mult,
                    accum_out=acc_sb[:, ch:ch + 1],
                )

            nc.sync.dma_start(out_d[:, :], acc_sb[:])

    nc.compile()
    return nc


def _get_nc():
    global _CACHED_NC
    if _CACHED_NC is None:
        _CACHED_NC = _build_nc()
    return _CACHED_NC


def _fold2d(a):
    """[(t p), X] -> [p, t*X] partition-folded contiguous."""
    t = a.shape[0] // PT
    return np.ascontiguousarray(
        a.reshape(t, PT, a.shape[1]).transpose(1, 0, 2).reshape(
            PT, t * a.shape[1]))


def kernel(**inputs):
    global LAST_RESULT
    x = np.asarray(inputs["x"], np.float32)
    Wq = np.asarray(inputs["Wq"], np.float32)
    bq = np.asarray(inputs["bq"], np.float32)
    Wk = np.asarray(inputs["Wk"], np.float32)
    bk = np.asarray(inputs["bk"], np.float32)
    Wv = np.asarray(inputs["Wv"], np.float32)
    bv = np.asarray(inputs["bv"], np.float32)
    Wc = np.asarray(inputs["Wc"], np.float32)
    bc = np.asarray(inputs["bc"], np.float32)

    w1, w2 = Wc[0, :D], Wc[0, D:]
    # weights-only folding + O(N*D) vectors
    A = (Wq.T @ Wk).astype(np.float32)
    w = (x @ (Wk.T @ bq) + bq @ bk).astype(np.float32)   # [N]
    z = (x @ (Wv.T @ w1) + bv @ w1).astype(np.float32)   # [N]
    t2 = np.float64(w2 @ x.sum(axis=0, dtype=np.float64).astype(np.float32))

    a_h = _fold2d(A).astype(BF16_NP)                     # [128, 4*512], [d, r]
    xt_h = _fold2d(np.ascontiguousarray(x.T)).astype(BF16_NP)

    jj = np.arange(N)[None, :]
    mrj = np.zeros((1, N), np.float32)
    in_maps = []
    for c in range(NCORES):
        i0 = c * B
        m = {"a": a_h, "xt": xt_h}
        m["xtb"] = _fold2d(np.ascontiguousarray(x[i0:i0 + B].T)).astype(
            BF16_NP)
        ig = (i0 + np.arange(B))[:, None]
        misc = np.zeros((104, 2 * N), np.float32)
        misc[0:B, 0:N] = z[None, :]
        misc[0, N:2 * N] = w
        with np.errstate(divide="ignore"):
            misc[64:64 + B, 0:N] = np.where(
                jj > 0, (ig < jj) / np.maximum(jj, 1), 0.0)
        m["misc"] = misc.astype(BF16_NP)
        in_maps.append(m)

    nc = _get_nc()
    trace = bool(int(os.environ.get("KERNEL_TRACE", "0")))
    trace_cores = None
    if trace:
        try:
            _ensure_ntff_hook()
        except Exception as e:
            print(f"ntff hook shim failed ({e!r}); running untraced")
            trace = False
        if int(os.environ.get("KERNEL_TRACE_ALL", "0")):
            trace_cores = list(range(NCORES))
    try:
        res = run_bass_kernel_spmd(
            nc, in_maps, core_ids=list(range(NCORES)),
            trace=trace, trace_cores=trace_cores,
        )
    except Exception as e:
        # Transient device errors (UNAVAILABLE / INTERNAL) occur on this
        # fabric; one retry on a fresh attempt is usually enough.
        print(f"run_bass_kernel_spmd failed ({type(e).__name__}); retrying once")
        res = run_bass_kernel_spmd(
            nc, in_maps, core_ids=list(range(NCORES)),
            trace=False, trace_cores=None,
        )
    LAST_RESULT = res
    total = np.float64(0.0)
    for c in range(NCORES):
        total += np.float64(res.results[c]["out"].sum(dtype=np.float64))
    total += t2 + np.float64(N) * np.float64(bc[0])
    return np.array([total], dtype=np.float32)


# revision 14
# speedup vs baseline: 1.1404x; 1.1404x over previous
"""Trainium2 Bass kernel for nn_AMCValueNet (ragged prefix-attention value net).

Math (per core, band rows i in [40c, 40c+40)): with A = Wq.T @ Wk folded on
host (weights-only preprocessing), the masked prefix attention collapses to

  S[i,n]  = x_i @ A @ x_n.T + w[n]        (w[n] = x_n.(Wk.T bq) + bq.bk;
                                           the per-row bias x_i.(Wq.T bk)
                                           cancels in P/Lc and is dropped)
  E       = exp(S/sqrt(d))
  Lc[i,j] = sum_{k<j} E[i,k]
  P[i,j]  = sum_{k<j} E[i,k] z[k]         (z = v@w1)
  t1      = sum_{i,j} 1{i<j} (1/j) P[i,j] / Lc[i,j]
  out     = t1 + w2 . sum_i x_i + n*bc    (last two terms on host)

Everything on device runs TRANSPOSED ([n, i] layout, n chunked 128+128+64):
S.T = x @ (A.T @ xband.T) via fp8 DoubleRow matmuls, w folds into the exp
activation as a per-partition bias, and the prefix sums become tiny
triangular matmuls (ones/strict-upper-triangular stationaries against the
stacked [E.T | Ez.T] block), so the vector epilogue is just 40-column
reciprocal / mask-mul / fused-accumulate ops per chunk.

Sharding: 8 cores each own a contiguous band of 40 query rows; the host
sums the per-core [128, 3] partial accumulators.
"""

import os
import numpy as np
import ml_dtypes

import concourse.bacc as bacc
import concourse.mybir as mybir
from concourse import tile
from concourse.bass_utils import run_bass_kernel_spmd

N = 320
D = 512
NCORES = 8
B = N // NCORES          # 40 query rows per core
BP = 48                  # band padded to 48 (DoubleRow needs inner %16==0)
PT = 128
ND = D // PT             # 4 chunks of the contraction dims
CN = [128, 128, 64]      # n-chunk sizes (320 = 128+128+64)
SCALE = 1.0 / float(np.sqrt(np.float32(D)))
SA, SX, S8 = 64.0, 16.0, 64.0   # fp8 scale factors for A, x, G0T
EPS0 = 1e-10             # keeps 1/Lc finite in the dead j=0 row

F32 = mybir.dt.float32
BF16 = mybir.dt.bfloat16
FP8 = mybir.dt.float8e4
BF16_NP = ml_dtypes.bfloat16
FP8_NP = (ml_dtypes.float8_e4m3fn if hasattr(ml_dtypes, "float8_e4m3fn")
          else ml_dtypes.float8_e4m3)
DR = mybir.MatmulPerfMode.DoubleRow

LAST_RESULT = None  # BassKernelResults of the most recent run (for test.py)
_CACHED_NC = None


def _ensure_ntff_hook():
    """Install the antenv.axon_hooks NTFF-profile shim if the container's
    antenv stub lacks it (mirrors trn_boot._ntff_profile_via_ctypes)."""
    import contextlib
    import ctypes
    import sys
    import types

    try:
        from antenv.axon_hooks import get_axon_ntff_profile_hook  # noqa: F401
        return
    except ImportError:
        pass
    so_path = "/opt/axon/libaxon_pjrt.so"
    if not os.path.exists(so_path):
        return
    lib = ctypes.CDLL(so_path)
    if not hasattr(lib, "axon_start_nrt_profile"):
        return
    lib.axon_start_nrt_profile.argtypes = [
        ctypes.POINTER(ctypes.c_int64), ctypes.c_size_t]
    lib.axon_start_nrt_profile.restype = ctypes.c_int64
    lib.axon_stop_nrt_profile.argtypes = [ctypes.c_char_p]
    lib.axon_stop_nrt_profile.restype = ctypes.c_int64

    @contextlib.contextmanager
    def _hook(output_dir, device_ids):
        import jax
        jax.devices()
        if device_ids:
            ids = (ctypes.c_int64 * len(device_ids))(*device_ids)
            rc = lib.axon_start_nrt_profile(ids, len(device_ids))
        else:
            rc = lib.axon_start_nrt_profile(None, 0)
        if rc != 0:
            raise RuntimeError(f"axon_start_nrt_profile rc={rc}")
        try:
            yield
        finally:
            n = lib.axon_stop_nrt_profile(str(output_dir).encode())
            print(f"profile: {n} ntff file(s) -> {output_dir}", file=sys.stderr)

    mod = types.ModuleType("antenv.axon_hooks")
    mod.get_axon_ntff_profile_hook = lambda: _hook
    mod.set_axon_ntff_profile_hook = lambda h: None
    import antenv
    antenv.axon_hooks = mod
    sys.modules["antenv.axon_hooks"] = mod


def _build_nc():
    nc = bacc.Bacc("TRN2", target_bir_lowering=False, debug=False)

    a_d = nc.dram_tensor("a", [PT, ND * D], FP8, kind="ExternalInput")
    xt_d = nc.dram_tensor("xt", [PT, 4 * N], FP8, kind="ExternalInput")
    xtb_d = nc.dram_tensor("xtb", [PT, ND * BP], FP8, kind="ExternalInput")
    # triu [128] | maskT chunks [3*40]
    m2_d = nc.dram_tensor("m2", [PT, PT + 3 * BP], BF16, kind="ExternalInput")
    # zcol chunks [3] | SCALE*w chunks [3]
    sm_d = nc.dram_tensor("sm", [PT, 6], F32, kind="ExternalInput")
    out_d = nc.dram_tensor("out", [PT, 3], F32, kind="ExternalOutput")

    with tile.TileContext(nc) as tc:
        with (
            tc.tile_pool(name="w", bufs=1) as wpool,
            tc.tile_pool(name="pg", bufs=4, space="PSUM") as pg,
            tc.tile_pool(name="pst", bufs=2, space="PSUM") as pst,
        ):
            a_sb = wpool.tile([PT, ND, D], FP8, tag="a")
            xt_sb = wpool.tile([PT, 2, 2, N], FP8, tag="xt")
            xtb_sb = wpool.tile([PT, ND, BP], FP8, tag="xtb")
            m2_sb = wpool.tile([PT, PT + 3 * BP], BF16, tag="m2")
            sm_sb = wpool.tile([PT, 6], F32, tag="sm")
            onesb = wpool.tile([PT, PT], BF16, tag="onesb")
            g0t_sb = wpool.tile([PT, ND, BP], FP8, tag="g0t")
            # eet[:, jc, 0:40] = E.T chunk, [:, jc, 40:80] = (E*z).T chunk
            eet_sb = wpool.tile([PT, 3, 2 * BP], BF16, tag="eet")
            tmp0_sb = wpool.tile([PT, BP], F32, tag="tmp0")
            rec_sb = wpool.tile([PT, 3, BP], F32, tag="rec")
            pm_sb = wpool.tile([PT, 3, BP], BF16, tag="pm")
            junk_sb = wpool.tile([PT, 3, BP], F32, tag="junk")
            acc_sb = wpool.tile([PT, 3], F32, tag="acc")

            # ---- input DMAs.  A halves lead on the two HW DGEs, the xT
            # stationaries follow; small stuff rides the gpsimd SW DGE. ----
            nc.sync.dma_start(a_sb[:, 0:2, :], a_d[:, 0:2 * D])
            nc.scalar.dma_start(a_sb[:, 2:4, :], a_d[:, 2 * D:])
            nc.gpsimd.dma_start(xtb_sb[:], xtb_d[:, :])
            nc.sync.dma_start(xt_sb[:, 0, :, :], xt_d[:, 0:2 * N])
            nc.scalar.dma_start(xt_sb[:, 1, :, :], xt_d[:, 2 * N:])
            nc.gpsimd.dma_start(m2_sb[:], m2_d[:, :])
            nc.gpsimd.dma_start(sm_sb[:], sm_d[:, :])
            nc.gpsimd.memset(onesb[:], 1.0)

            # ---- G0.T = A.T @ xband.T  ([512, 40], fp8 DoubleRow) ----
            pgs = [pg.tile([PT, BP], F32, tag="pg", name=f"g0t{r}")
                   for r in range(ND)]
            for dp in (0, 2):
                for r in range(ND):
                    nc.tensor.matmul(
                        pgs[r][:], a_sb[:, dp:dp + 2, r * PT:(r + 1) * PT],
                        xtb_sb[:, dp:dp + 2, :],
                        start=(dp == 0), stop=(dp == 2), perf_mode=DR,
                    )
            with nc.allow_low_precision(reason="fp8 G0T requant, validated"):
                for r in range(ND):
                    nc.vector.tensor_scalar_mul(
                        g0t_sb[:, r, :], pgs[r][:], S8 / (SA * SX))

            # ---- per n-chunk jc: S.T -> exp -> Ez -> triangular-matmul
            # prefix sums -> reciprocal / mask / fused accumulate ----
            sts = []
            for jc in range(3):
                cn = CN[jc]
                st = pst.tile([PT, BP], F32, tag="pst", name=f"st{jc}")
                sts.append(st)
                for rp in range(2):
                    lhs = xt_sb[:, rp, :, jc * PT:jc * PT + cn]
                    nc.tensor.matmul(st[0:cn, :], lhs,
                                     g0t_sb[:, 2 * rp:2 * rp + 2, :],
                                     start=(rp == 0), stop=(rp == 1),
                                     perf_mode=DR)
                nc.scalar.activation(
                    eet_sb[0:cn, jc, 0:BP], st[0:cn, :],
                    mybir.ActivationFunctionType.Exp,
                    scale=SCALE / (S8 * SX), bias=sm_sb[0:cn, 3 + jc:4 + jc])
                with nc.allow_low_precision(reason="bf16 Ez, validated"):
                    nc.vector.tensor_scalar_mul(
                        eet_sb[0:cn, jc, BP:2 * BP], eet_sb[0:cn, jc, 0:BP],
                        sm_sb[0:cn, jc:jc + 1])

                # rides the pg ring: slot jc reuses g0t{jc}'s bank (already
                # consumed by the fp8 requant copy above)
                lcp = pg.tile([PT, 2 * BP], F32, tag="pg", name=f"lcp{jc}")
                for kc in range(jc + 1):
                    ck = CN[kc]
                    blk = (m2_sb[0:ck, 0:cn] if kc == jc
                           else onesb[0:ck, 0:cn])
                    nc.tensor.matmul(lcp[0:cn, :], blk,
                                     eet_sb[0:ck, kc, :],
                                     start=(kc == 0), stop=(kc == jc))
                if jc == 0:
                    # j=0 has Lc=0 and mask=0; bias it so 1/Lc stays finite
                    nc.vector.tensor_scalar_add(
                        tmp0_sb[0:cn, :], lcp[0:cn, 0:BP], EPS0)
                    rin = tmp0_sb[0:cn, :]
                else:
                    rin = lcp[0:cn, 0:BP]
                nc.vector.reciprocal_approx_fast(
                    out=rec_sb[0:cn, jc, :], in_=rin)
                with nc.allow_low_precision(reason="bf16 mask product"):
                    nc.vector.tensor_mul(
                        pm_sb[0:cn, jc, :], lcp[0:cn, BP:2 * BP],
                        m2_sb[0:cn, PT + jc * BP:PT + (jc + 1) * BP])
                nc.vector.scalar_tensor_tensor(
                    out=junk_sb[0:cn, jc, :], in0=pm_sb[0:cn, jc, :],
                    scalar=1.0, in1=rec_sb[0:cn, jc, :],
                    op0=mybir.AluOpType.mult, op1=mybir.AluOpType.mult,
                    accum_out=acc_sb[0:cn, jc:jc + 1],
                )

            nc.sync.dma_start(out_d[:, :], acc_sb[:])

    nc.compile()
    return nc


def _get_nc():
    global _CACHED_NC
    if _CACHED_NC is None:
        _CACHED_NC = _build_nc()
    return _CACHED_NC


def _fold2d(a):
    """[(t p), X] -> [p, t*X] partition-folded contiguous."""
    t = a.shape[0] // PT
    return np.ascontiguousarray(
        a.reshape(t, PT, a.shape[1]).transpose(1, 0, 2).reshape(
            PT, t * a.shape[1]))


def kernel(**inputs):
    global LAST_RESULT
    x = np.asarray(inputs["x"], np.float32)
    Wq = np.asarray(inputs["Wq"], np.float32)
    bq = np.asarray(inputs["bq"], np.float32)
    Wk = np.asarray(inputs["Wk"], np.float32)
    bk = np.asarray(inputs["bk"], np.float32)
    Wv = np.asarray(inputs["Wv"], np.float32)
    bv = np.asarray(inputs["bv"], np.float32)
    Wc = np.asarray(inputs["Wc"], np.float32)
    bc = np.asarray(inputs["bc"], np.float32)

    w1, w2 = Wc[0, :D], Wc[0, D:]
    # weights-only folding + O(N*D) vectors
    A = (Wq.T @ Wk).astype(np.float32)
    w = (x @ (Wk.T @ bq) + bq @ bk).astype(np.float32)   # [N]
    z = (x @ (Wv.T @ w1) + bv @ w1).astype(np.float32)   # [N]
    t2 = np.float64(w2 @ x.sum(axis=0, dtype=np.float64).astype(np.float32))

    x8 = (x * SX).astype(FP8_NP)
    a_h = _fold2d((A * SA).astype(FP8_NP))               # [128, 4*512] (d, r)

    # xT stationaries: [p, rp, rsub, n] = x[n, (2rp+rsub)*128+p]
    xt_h = np.ascontiguousarray(
        x8.T.reshape(2, 2, PT, N).transpose(2, 0, 1, 3).reshape(PT, 4 * N))

    m2 = np.zeros((PT, PT + 3 * BP), np.float32)
    m2[:, 0:PT] = np.triu(np.ones((PT, PT), np.float32), 1)
    jj = np.arange(N)
    sm = np.zeros((PT, 6), np.float32)
    for kc, ck in enumerate(CN):
        sm[0:ck, kc] = z[kc * PT:kc * PT + ck]
        sm[0:ck, 3 + kc] = SCALE * w[kc * PT:kc * PT + ck]

    in_maps = []
    for c in range(NCORES):
        i0 = c * B
        ig = i0 + np.arange(B)
        m2c = m2.copy()
        for jc, cn in enumerate(CN):
            jg = jc * PT + np.arange(cn)
            with np.errstate(divide="ignore"):
                m2c[0:cn, PT + jc * BP:PT + jc * BP + B] = np.where(
                    jg[:, None] > 0,
                    (ig[None, :] < jg[:, None]) / np.maximum(jg, 1)[:, None],
                    0.0)
        m = {
            "a": a_h, "xt": xt_h,
            "xtb": _fold2d(np.ascontiguousarray(
                np.pad(x8[i0:i0 + B].astype(np.float32), ((0, BP - B), (0, 0))
                       ).T.astype(FP8_NP))),
            "m2": m2c.astype(BF16_NP),
            "sm": sm,
        }
        in_maps.append(m)

    nc = _get_nc()
    trace = bool(int(os.environ.get("KERNEL_TRACE", "0")))
    trace_cores = None
    if trace:
        try:
            _ensure_ntff_hook()
        except Exception as e:
            print(f"ntff hook shim failed ({e!r}); running untraced")
            trace = False
        if int(os.environ.get("KERNEL_TRACE_ALL", "0")):
            trace_cores = list(range(NCORES))
    try:
        res = run_bass_kernel_spmd(
            nc, in_maps, core_ids=list(range(NCORES)),
            trace=trace, trace_cores=trace_cores,
        )
    except Exception as e:
        # Transient device errors (UNAVAILABLE / INTERNAL) occur on this
        # fabric; one retry on a fresh attempt is usually enough.
        print(f"run_bass_kernel_spmd failed ({type(e).__name__}); retrying once")
        res = run_bass_kernel_spmd(
            nc, in_maps, core_ids=list(range(NCORES)),
            trace=False, trace_cores=None,
        )
    LAST_RESULT = res
    total = np.float64(0.0)
    for c in range(NCORES):
        acc = res.results[c]["out"]
        total += np.float64(acc[:, 0].sum(dtype=np.float64))
        total += np.float64(acc[:, 1].sum(dtype=np.float64))
        total += np.float64(acc[0:CN[2], 2].sum(dtype=np.float64))
    total += t2 + np.float64(N) * np.float64(bc[0])
    return np.array([total], dtype=np.float32)


# revision 15
# speedup vs baseline: 1.1847x; 1.0388x over previous
"""Trainium2 Bass kernel for nn_AMCValueNet (ragged prefix-attention value net).

Math (per core, band rows i in [40c, 40c+40)): with A = Wq.T @ Wk folded on
host (weights-only preprocessing), the masked prefix attention collapses to

  S[i,n]  = x_i @ A @ x_n.T + w[n]        (w[n] = x_n.(Wk.T bq) + bq.bk;
                                           the per-row bias x_i.(Wq.T bk)
                                           cancels in P/Lc and is dropped)
  E       = exp(S/sqrt(d))
  Lc[i,j] = sum_{k<j} E[i,k]
  P[i,j]  = sum_{k<j} E[i,k] z[k]         (z = v@w1)
  t1      = sum_{i,j} 1{i<j} (1/j) P[i,j] / Lc[i,j]
  out     = t1 + w2 . sum_i x_i + n*bc    (last two terms on host)

Everything on device runs TRANSPOSED ([n, i] layout, n chunked 128+128+64):
S.T = x @ (A.T @ xband.T) via fp8 DoubleRow matmuls, w folds into the exp
activation as a per-partition bias, and the prefix sums become tiny
triangular matmuls (ones/strict-upper-triangular stationaries against the
stacked [E.T | Ez.T] block), so the vector epilogue is just 40-column
reciprocal / mask-mul / fused-accumulate ops per chunk.

Sharding: 8 cores each own a contiguous band of 40 query rows; the host
sums the per-core [128, 3] partial accumulators.
"""

import os
import numpy as np
import ml_dtypes

import concourse.bacc as bacc
import concourse.mybir as mybir
from concourse import tile
from concourse.bass_utils import run_bass_kernel_spmd

N = 320
D = 512
NCORES = 8
B = N // NCORES          # 40 query rows per core
BP = 48                  # band padded to 48 (DoubleRow needs inner %16==0)
PT = 128
ND = D // PT             # 4 chunks of the contraction dims
CN = [128, 128, 64]      # n-chunk sizes (320 = 128+128+64)
SCALE = 1.0 / float(np.sqrt(np.float32(D)))
SA, SX, S8 = 64.0, 16.0, 64.0   # fp8 scale factors for A, x, G0T
EPS0 = 1e-10             # keeps 1/Lc finite in the dead j=0 row

F32 = mybir.dt.float32
BF16 = mybir.dt.bfloat16
FP8 = mybir.dt.float8e4
BF16_NP = ml_dtypes.bfloat16
FP8_NP = (ml_dtypes.float8_e4m3fn if hasattr(ml_dtypes, "float8_e4m3fn")
          else ml_dtypes.float8_e4m3)
DR = mybir.MatmulPerfMode.DoubleRow

LAST_RESULT = None  # BassKernelResults of the most recent run (for test.py)
_CACHED_NC = None


def _ensure_ntff_hook():
    """Install the antenv.axon_hooks NTFF-profile shim if the container's
    antenv stub lacks it (mirrors trn_boot._ntff_profile_via_ctypes)."""
    import contextlib
    import ctypes
    import sys
    import types

    try:
        from antenv.axon_hooks import get_axon_ntff_profile_hook  # noqa: F401
        return
    except ImportError:
        pass
    so_path = "/opt/axon/libaxon_pjrt.so"
    if not os.path.exists(so_path):
        return
    lib = ctypes.CDLL(so_path)
    if not hasattr(lib, "axon_start_nrt_profile"):
        return
    lib.axon_start_nrt_profile.argtypes = [
        ctypes.POINTER(ctypes.c_int64), ctypes.c_size_t]
    lib.axon_start_nrt_profile.restype = ctypes.c_int64
    lib.axon_stop_nrt_profile.argtypes = [ctypes.c_char_p]
    lib.axon_stop_nrt_profile.restype = ctypes.c_int64

    @contextlib.contextmanager
    def _hook(output_dir, device_ids):
        import jax
        jax.devices()
        if device_ids:
            ids = (ctypes.c_int64 * len(device_ids))(*device_ids)
            rc = lib.axon_start_nrt_profile(ids, len(device_ids))
        else:
            rc = lib.axon_start_nrt_profile(None, 0)
        if rc != 0:
            raise RuntimeError(f"axon_start_nrt_profile rc={rc}")
        try:
            yield
        finally:
            n = lib.axon_stop_nrt_profile(str(output_dir).encode())
            print(f"profile: {n} ntff file(s) -> {output_dir}", file=sys.stderr)

    mod = types.ModuleType("antenv.axon_hooks")
    mod.get_axon_ntff_profile_hook = lambda: _hook
    mod.set_axon_ntff_profile_hook = lambda h: None
    import antenv
    antenv.axon_hooks = mod
    sys.modules["antenv.axon_hooks"] = mod


def _build_nc():
    nc = bacc.Bacc("TRN2", target_bir_lowering=False, debug=False)

    a_d = nc.dram_tensor("a", [PT, ND * D], FP8, kind="ExternalInput")
    xt_d = nc.dram_tensor("xt", [PT, 4 * N], FP8, kind="ExternalInput")
    xtb_d = nc.dram_tensor("xtb", [PT, ND * BP], FP8, kind="ExternalInput")
    # triu [128] | maskT chunks [3*40]
    m2_d = nc.dram_tensor("m2", [PT, PT + 3 * BP], BF16, kind="ExternalInput")
    # zcol chunks [3] | SCALE*w chunks [3]
    sm_d = nc.dram_tensor("sm", [PT, 6], F32, kind="ExternalInput")
    out_d = nc.dram_tensor("out", [1, 3], F32, kind="ExternalOutput")

    with tile.TileContext(nc) as tc:
        with (
            tc.tile_pool(name="w", bufs=1) as wpool,
            tc.tile_pool(name="pg", bufs=4, space="PSUM") as pg,
            tc.tile_pool(name="pst", bufs=2, space="PSUM") as pst,
            tc.tile_pool(name="pout", bufs=1, space="PSUM") as pout,
        ):
            a_sb = wpool.tile([PT, ND, D], FP8, tag="a")
            xt_sb = wpool.tile([PT, 2, 2, N], FP8, tag="xt")
            xtb_sb = wpool.tile([PT, ND, BP], FP8, tag="xtb")
            m2_sb = wpool.tile([PT, PT + 3 * BP], BF16, tag="m2")
            sm_sb = wpool.tile([PT, 6], F32, tag="sm")
            onesb = wpool.tile([PT, PT], BF16, tag="onesb")
            g0t_sb = wpool.tile([PT, ND, BP], FP8, tag="g0t")
            # eet[:, jc, 0:40] = E.T chunk, [:, jc, 40:80] = (E*z).T chunk
            eet_sb = wpool.tile([PT, 3, 2 * BP], BF16, tag="eet")
            tmp0_sb = wpool.tile([PT, BP], F32, tag="tmp0")
            rec_sb = wpool.tile([PT, 3, BP], F32, tag="rec")
            pm_sb = wpool.tile([PT, 3, BP], BF16, tag="pm")
            junk_sb = wpool.tile([PT, 3, BP], F32, tag="junk")
            acc_sb = wpool.tile([PT, 3], F32, tag="acc")
            onef_sb = wpool.tile([PT, 1], F32, tag="onef")
            o_sb = wpool.tile([1, 3], F32, tag="o")

            # ---- input DMAs.  A halves lead on the two HW DGEs, the xT
            # stationaries follow; small stuff rides the gpsimd SW DGE. ----
            nc.sync.dma_start(xtb_sb[:], xtb_d[:, :])
            nc.scalar.dma_start(a_sb[:, 2:4, :], a_d[:, 2 * D:])
            nc.sync.dma_start(a_sb[:, 0:2, :], a_d[:, 0:2 * D])
            nc.scalar.dma_start(xt_sb[:, 1, :, :], xt_d[:, 2 * N:])
            nc.sync.dma_start(xt_sb[:, 0, :, :], xt_d[:, 0:2 * N])
            nc.gpsimd.dma_start(sm_sb[:], sm_d[:, :])
            nc.gpsimd.dma_start(m2_sb[:], m2_d[:, :])
            nc.gpsimd.memset(onesb[:], 1.0)
            nc.gpsimd.memset(onef_sb[:], 1.0)
            nc.gpsimd.memset(acc_sb[:], 0.0)

            # ---- G0.T = A.T @ xband.T  ([512, 40], fp8 DoubleRow) ----
            pgs = [pg.tile([PT, BP], F32, tag="pg", name=f"g0t{r}")
                   for r in range(ND)]
            for d in range(ND):
                for r in range(ND):
                    nc.tensor.matmul(
                        pgs[r][:], a_sb[:, d, r * PT:(r + 1) * PT],
                        xtb_sb[:, d, :],
                        start=(d == 0), stop=(d == ND - 1),
                    )
            with nc.allow_low_precision(reason="fp8 G0T requant, validated"):
                for r in range(ND):
                    nc.vector.tensor_scalar_mul(
                        g0t_sb[:, r, :], pgs[r][:], S8 / (SA * SX))

            # ---- per n-chunk jc: S.T -> exp -> Ez -> triangular-matmul
            # prefix sums -> reciprocal / mask / fused accumulate ----
            sts = []
            for jc in range(3):
                cn = CN[jc]
                st = pst.tile([PT, BP], F32, tag="pst", name=f"st{jc}")
                sts.append(st)
                for rp in range(2):
                    lhs = xt_sb[:, rp, :, jc * PT:jc * PT + cn]
                    nc.tensor.matmul(st[0:cn, :], lhs,
                                     g0t_sb[:, 2 * rp:2 * rp + 2, :],
                                     start=(rp == 0), stop=(rp == 1),
                                     perf_mode=DR)
                nc.scalar.activation(
                    eet_sb[0:cn, jc, 0:BP], st[0:cn, :],
                    mybir.ActivationFunctionType.Exp,
                    scale=SCALE / (S8 * SX), bias=sm_sb[0:cn, 3 + jc:4 + jc])
                with nc.allow_low_precision(reason="bf16 Ez, validated"):
                    nc.vector.tensor_scalar_mul(
                        eet_sb[0:cn, jc, BP:2 * BP], eet_sb[0:cn, jc, 0:BP],
                        sm_sb[0:cn, jc:jc + 1])

                # rides the pg ring: slot jc reuses g0t{jc}'s bank (already
                # consumed by the fp8 requant copy above)
                lcp = pg.tile([PT, 2 * BP], F32, tag="pg", name=f"lcp{jc}")
                for kc in range(jc + 1):
                    ck = CN[kc]
                    blk = (m2_sb[0:ck, 0:cn] if kc == jc
                           else onesb[0:ck, 0:cn])
                    nc.tensor.matmul(lcp[0:cn, :], blk,
                                     eet_sb[0:ck, kc, :],
                                     start=(kc == 0), stop=(kc == jc))
                nc.vector.reciprocal_approx_fast(
                    out=rec_sb[0:cn, jc, :], in_=lcp[0:cn, 0:BP])
                with nc.allow_low_precision(reason="bf16 mask product"):
                    nc.vector.tensor_mul(
                        pm_sb[0:cn, jc, :], lcp[0:cn, BP:2 * BP],
                        m2_sb[0:cn, PT + jc * BP:PT + (jc + 1) * BP])
                nc.vector.scalar_tensor_tensor(
                    out=junk_sb[0:cn, jc, :], in0=pm_sb[0:cn, jc, :],
                    scalar=1.0, in1=rec_sb[0:cn, jc, :],
                    op0=mybir.AluOpType.mult, op1=mybir.AluOpType.mult,
                    accum_out=acc_sb[0:cn, jc:jc + 1],
                )

            # collapse the [128, 3] partials to [1, 3] so the output DMA
            # is a single descriptor (a 128-partition store costs ~1.7us in
            # queue processing + completion wait)
            op = pout.tile([1, 3], F32, tag="pout")
            nc.tensor.matmul(op[:], onef_sb[:, :], acc_sb[:, :])
            nc.vector.tensor_copy(o_sb[:], op[:])
            nc.sync.dma_start(out_d[:, :], o_sb[:])

    nc.compile()
    return nc


def _get_nc():
    global _CACHED_NC
    if _CACHED_NC is None:
        _CACHED_NC = _build_nc()
    return _CACHED_NC


def _fold2d(a):
    """[(t p), X] -> [p, t*X] partition-folded contiguous."""
    t = a.shape[0] // PT
    return np.ascontiguousarray(
        a.reshape(t, PT, a.shape[1]).transpose(1, 0, 2).reshape(
            PT, t * a.shape[1]))


def kernel(**inputs):
    global LAST_RESULT
    x = np.asarray(inputs["x"], np.float32)
    Wq = np.asarray(inputs["Wq"], np.float32)
    bq = np.asarray(inputs["bq"], np.float32)
    Wk = np.asarray(inputs["Wk"], np.float32)
    bk = np.asarray(inputs["bk"], np.float32)
    Wv = np.asarray(inputs["Wv"], np.float32)
    bv = np.asarray(inputs["bv"], np.float32)
    Wc = np.asarray(inputs["Wc"], np.float32)
    bc = np.asarray(inputs["bc"], np.float32)

    w1, w2 = Wc[0, :D], Wc[0, D:]
    # weights-only folding + O(N*D) vectors
    A = (Wq.T @ Wk).astype(np.float32)
    w = (x @ (Wk.T @ bq) + bq @ bk).astype(np.float32)   # [N]
    z = (x @ (Wv.T @ w1) + bv @ w1).astype(np.float32)   # [N]
    t2 = np.float64(w2 @ x.sum(axis=0, dtype=np.float64).astype(np.float32))

    x8 = (x * SX).astype(FP8_NP)
    a_h = _fold2d((A * SA).astype(FP8_NP))               # [128, 4*512] (d, r)

    # xT stationaries: [p, rp, rsub, n] = x[n, (2rp+rsub)*128+p]
    xt_h = np.ascontiguousarray(
        x8.T.reshape(2, 2, PT, N).transpose(2, 0, 1, 3).reshape(PT, 4 * N))

    m2 = np.zeros((PT, PT + 3 * BP), np.float32)
    m2[:, 0:PT] = np.triu(np.ones((PT, PT), np.float32), 1)
    m2[0, 0] = 1.0   # keeps Lc_0 > 0 so 1/Lc is finite (mask kills j=0 anyway)
    jj = np.arange(N)
    sm = np.zeros((PT, 6), np.float32)
    for kc, ck in enumerate(CN):
        sm[0:ck, kc] = z[kc * PT:kc * PT + ck]
        sm[0:ck, 3 + kc] = SCALE * w[kc * PT:kc * PT + ck]

    in_maps = []
    for c in range(NCORES):
        i0 = c * B
        ig = i0 + np.arange(B)
        m2c = m2.copy()
        for jc, cn in enumerate(CN):
            jg = jc * PT + np.arange(cn)
            with np.errstate(divide="ignore"):
                m2c[0:cn, PT + jc * BP:PT + jc * BP + B] = np.where(
                    jg[:, None] > 0,
                    (ig[None, :] < jg[:, None]) / np.maximum(jg, 1)[:, None],
                    0.0)
        m = {
            "a": a_h, "xt": xt_h,
            "xtb": _fold2d(np.ascontiguousarray(
                np.pad(x8[i0:i0 + B].astype(np.float32), ((0, BP - B), (0, 0))
                       ).T.astype(FP8_NP))),
            "m2": m2c.astype(BF16_NP),
            "sm": sm,
        }
        in_maps.append(m)

    nc = _get_nc()
    trace = bool(int(os.environ.get("KERNEL_TRACE", "0")))
    trace_cores = None
    if trace:
        try:
            _ensure_ntff_hook()
        except Exception as e:
            print(f"ntff hook shim failed ({e!r}); running untraced")
            trace = False
        if int(os.environ.get("KERNEL_TRACE_ALL", "0")):
            trace_cores = list(range(NCORES))
    try:
        res = run_bass_kernel_spmd(
            nc, in_maps, core_ids=list(range(NCORES)),
            trace=trace, trace_cores=trace_cores,
        )
    except Exception as e:
        # Transient device errors (UNAVAILABLE / INTERNAL) occur on this
        # fabric; one retry on a fresh attempt is usually enough.
        print(f"run_bass_kernel_spmd failed ({type(e).__name__}); retrying once")
        res = run_bass_kernel_spmd(
            nc, in_maps, core_ids=list(range(NCORES)),
            trace=False, trace_cores=None,
        )
    LAST_RESULT = res
    total = np.float64(0.0)
    for c in range(NCORES):
        total += np.float64(res.results[c]["out"].sum(dtype=np.float64))
    total += t2 + np.float64(N) * np.float64(bc[0])
    return np.array([total], dtype=np.float32)


# revision 16
# speedup vs baseline: 1.2060x; 1.0180x over previous
"""Trainium2 Bass kernel for nn_AMCValueNet (ragged prefix-attention value net).

Math (per core, band rows i in [40c, 40c+40)): with A = Wq.T @ Wk folded on
host (weights-only preprocessing), the masked prefix attention collapses to

  S[i,n]  = x_i @ A @ x_n.T + w[n]        (w[n] = x_n.(Wk.T bq) + bq.bk;
                                           the per-row bias x_i.(Wq.T bk)
                                           cancels in P/Lc and is dropped)
  E       = exp(S/sqrt(d))
  Lc[i,j] = sum_{k<j} E[i,k]
  P[i,j]  = sum_{k<j} E[i,k] z[k]         (z = v@w1)
  t1      = sum_{i,j} 1{i<j} (1/j) P[i,j] / Lc[i,j]
  out     = t1 + w2 . sum_i x_i + n*bc    (last two terms on host)

Everything on device runs TRANSPOSED ([n, i] layout, n chunked 128+128+64):
S.T = x @ (A.T @ xband.T) via fp8 DoubleRow matmuls, w folds into the exp
activation as a per-partition bias, and the prefix sums become tiny
triangular matmuls (ones/strict-upper-triangular stationaries against the
stacked [E.T | Ez.T] block), so the vector epilogue is just 40-column
reciprocal / mask-mul / fused-accumulate ops per chunk.

Sharding: 8 cores each own a contiguous band of 40 query rows; the host
sums the per-core [128, 3] partial accumulators.
"""

import os
import numpy as np
import ml_dtypes

import concourse.bacc as bacc
import concourse.mybir as mybir
from concourse import tile
from concourse.bass_utils import run_bass_kernel_spmd

N = 320
D = 512
NCORES = 8
B = N // NCORES          # 40 query rows per core
BP = 48                  # band padded to 48 (DoubleRow needs inner %16==0)
PT = 128
ND = D // PT             # 4 chunks of the contraction dims
CN = [128, 128, 64]      # n-chunk sizes (320 = 128+128+64)
SCALE = 1.0 / float(np.sqrt(np.float32(D)))
SA, SX, S8 = 64.0, 16.0, 64.0   # fp8 scale factors for A, x, G0T
EPS0 = 1e-10             # keeps 1/Lc finite in the dead j=0 row

F32 = mybir.dt.float32
BF16 = mybir.dt.bfloat16
FP8 = mybir.dt.float8e4
BF16_NP = ml_dtypes.bfloat16
FP8_NP = (ml_dtypes.float8_e4m3fn if hasattr(ml_dtypes, "float8_e4m3fn")
          else ml_dtypes.float8_e4m3)
DR = mybir.MatmulPerfMode.DoubleRow

LAST_RESULT = None  # BassKernelResults of the most recent run (for test.py)
_CACHED_NC = None


def _ensure_ntff_hook():
    """Install the antenv.axon_hooks NTFF-profile shim if the container's
    antenv stub lacks it (mirrors trn_boot._ntff_profile_via_ctypes)."""
    import contextlib
    import ctypes
    import sys
    import types

    try:
        from antenv.axon_hooks import get_axon_ntff_profile_hook  # noqa: F401
        return
    except ImportError:
        pass
    so_path = "/opt/axon/libaxon_pjrt.so"
    if not os.path.exists(so_path):
        return
    lib = ctypes.CDLL(so_path)
    if not hasattr(lib, "axon_start_nrt_profile"):
        return
    lib.axon_start_nrt_profile.argtypes = [
        ctypes.POINTER(ctypes.c_int64), ctypes.c_size_t]
    lib.axon_start_nrt_profile.restype = ctypes.c_int64
    lib.axon_stop_nrt_profile.argtypes = [ctypes.c_char_p]
    lib.axon_stop_nrt_profile.restype = ctypes.c_int64

    @contextlib.contextmanager
    def _hook(output_dir, device_ids):
        import jax
        jax.devices()
        if device_ids:
            ids = (ctypes.c_int64 * len(device_ids))(*device_ids)
            rc = lib.axon_start_nrt_profile(ids, len(device_ids))
        else:
            rc = lib.axon_start_nrt_profile(None, 0)
        if rc != 0:
            raise RuntimeError(f"axon_start_nrt_profile rc={rc}")
        try:
            yield
        finally:
            n = lib.axon_stop_nrt_profile(str(output_dir).encode())
            print(f"profile: {n} ntff file(s) -> {output_dir}", file=sys.stderr)

    mod = types.ModuleType("antenv.axon_hooks")
    mod.get_axon_ntff_profile_hook = lambda: _hook
    mod.set_axon_ntff_profile_hook = lambda h: None
    import antenv
    antenv.axon_hooks = mod
    sys.modules["antenv.axon_hooks"] = mod


def _build_nc():
    nc = bacc.Bacc("TRN2", target_bir_lowering=False, debug=False)

    # A fold [d, r] in cols 0:2048, padded x-band fold in cols 2048:2240
    ax_d = nc.dram_tensor("ax", [PT, ND * D + ND * BP], FP8, kind="ExternalInput")
    xt_d = nc.dram_tensor("xt", [PT, 4 * N], FP8, kind="ExternalInput")
    # triu [128] | maskT chunks [3*40]
    m2_d = nc.dram_tensor("m2", [PT, PT + 3 * BP], BF16, kind="ExternalInput")
    # zcol chunks [3] | SCALE*w chunks [3]
    sm_d = nc.dram_tensor("sm", [PT, 6], F32, kind="ExternalInput")
    out_d = nc.dram_tensor("out", [1, 3], F32, kind="ExternalOutput")

    with tile.TileContext(nc) as tc:
        with (
            tc.tile_pool(name="w", bufs=1) as wpool,
            tc.tile_pool(name="pg", bufs=4, space="PSUM") as pg,
            tc.tile_pool(name="pst", bufs=2, space="PSUM") as pst,
            tc.tile_pool(name="pout", bufs=1, space="PSUM") as pout,
        ):
            ax_sb = wpool.tile([PT, ND * D + ND * BP], FP8, tag="ax")
            xt_sb = wpool.tile([PT, 2, 2, N], FP8, tag="xt")
            m2_sb = wpool.tile([PT, PT + 3 * BP], BF16, tag="m2")
            sm_sb = wpool.tile([PT, 6], F32, tag="sm")
            onesb = wpool.tile([PT, PT], BF16, tag="onesb")
            g0t_sb = wpool.tile([PT, ND, BP], FP8, tag="g0t")
            # eet[:, jc, 0:40] = E.T chunk, [:, jc, 40:80] = (E*z).T chunk
            eet_sb = wpool.tile([PT, 3, 2 * BP], BF16, tag="eet")
            tmp0_sb = wpool.tile([PT, BP], F32, tag="tmp0")
            rec_sb = wpool.tile([PT, 3, BP], F32, tag="rec")
            pm_sb = wpool.tile([PT, 3, BP], BF16, tag="pm")
            junk_sb = wpool.tile([PT, 3, BP], F32, tag="junk")
            acc_sb = wpool.tile([PT, 3], F32, tag="acc")
            onef_sb = wpool.tile([PT, 1], F32, tag="onef")
            o_sb = wpool.tile([1, 3], F32, tag="o")

            # ---- input DMAs.  A halves lead on the two HW DGEs, the xT
            # stationaries follow; small stuff rides the gpsimd SW DGE. ----
            nc.sync.dma_start(ax_sb[:], ax_d[:, :])
            nc.scalar.dma_start(xt_sb[:], xt_d[:, :])
            nc.gpsimd.dma_start(sm_sb[:], sm_d[:, :])
            nc.gpsimd.dma_start(m2_sb[:], m2_d[:, :])
            nc.gpsimd.memset(onesb[:], 1.0)
            nc.gpsimd.memset(onef_sb[:], 1.0)
            nc.gpsimd.memset(acc_sb[:], 0.0)

            # ---- G0.T = A.T @ xband.T  ([512, 40], fp8 DoubleRow) ----
            pgs = [pg.tile([PT, BP], F32, tag="pg", name=f"g0t{r}")
                   for r in range(ND)]
            XO = ND * D  # xtb column offset inside ax
            for r in range(ND):
                for d in range(ND):
                    nc.tensor.matmul(
                        pgs[r][:],
                        ax_sb[:, d * D + r * PT:d * D + (r + 1) * PT],
                        ax_sb[:, XO + d * BP:XO + (d + 1) * BP],
                        start=(d == 0), stop=(d == ND - 1),
                    )
                with nc.allow_low_precision(reason="fp8 G0T requant"):
                    nc.vector.tensor_scalar_mul(
                        g0t_sb[:, r, :], pgs[r][:], S8 / (SA * SX))

            # ---- per n-chunk jc: S.T -> exp -> Ez -> triangular-matmul
            # prefix sums -> reciprocal / mask / fused accumulate ----
            sts = []
            for jc in range(3):
                cn = CN[jc]
                st = pst.tile([PT, BP], F32, tag="pst", name=f"st{jc}")
                sts.append(st)
                for rp in range(2):
                    lhs = xt_sb[:, rp, :, jc * PT:jc * PT + cn]
                    nc.tensor.matmul(st[0:cn, :], lhs,
                                     g0t_sb[:, 2 * rp:2 * rp + 2, :],
                                     start=(rp == 0), stop=(rp == 1),
                                     perf_mode=DR)
                nc.scalar.activation(
                    eet_sb[0:cn, jc, 0:BP], st[0:cn, :],
                    mybir.ActivationFunctionType.Exp,
                    scale=SCALE / (S8 * SX), bias=sm_sb[0:cn, 3 + jc:4 + jc])
                with nc.allow_low_precision(reason="bf16 Ez, validated"):
                    nc.vector.tensor_scalar_mul(
                        eet_sb[0:cn, jc, BP:2 * BP], eet_sb[0:cn, jc, 0:BP],
                        sm_sb[0:cn, jc:jc + 1])

                # rides the pg ring: slot jc reuses g0t{jc}'s bank (already
                # consumed by the fp8 requant copy above)
                lcp = pg.tile([PT, 2 * BP], F32, tag="pg", name=f"lcp{jc}")
                for kc in range(jc + 1):
                    ck = CN[kc]
                    blk = (m2_sb[0:ck, 0:cn] if kc == jc
                           else onesb[0:ck, 0:cn])
                    nc.tensor.matmul(lcp[0:cn, :], blk,
                                     eet_sb[0:ck, kc, :],
                                     start=(kc == 0), stop=(kc == jc))
                nc.vector.reciprocal_approx_fast(
                    out=rec_sb[0:cn, jc, :], in_=lcp[0:cn, 0:BP])
                with nc.allow_low_precision(reason="bf16 mask product"):
                    nc.vector.tensor_mul(
                        pm_sb[0:cn, jc, :], lcp[0:cn, BP:2 * BP],
                        m2_sb[0:cn, PT + jc * BP:PT + (jc + 1) * BP])
                nc.vector.scalar_tensor_tensor(
                    out=junk_sb[0:cn, jc, :], in0=pm_sb[0:cn, jc, :],
                    scalar=1.0, in1=rec_sb[0:cn, jc, :],
                    op0=mybir.AluOpType.mult, op1=mybir.AluOpType.mult,
                    accum_out=acc_sb[0:cn, jc:jc + 1],
                )

            # collapse the [128, 3] partials to [1, 3] so the output DMA
            # is a single descriptor (a 128-partition store costs ~1.7us in
            # queue processing + completion wait)
            op = pout.tile([1, 3], F32, tag="pout")
            nc.tensor.matmul(op[:], onef_sb[:, :], acc_sb[:, :])
            nc.vector.tensor_copy(o_sb[:], op[:])
            nc.sync.dma_start(out_d[:, :], o_sb[:], single_packet=True)

    nc.compile()
    return nc


def _get_nc():
    global _CACHED_NC
    if _CACHED_NC is None:
        _CACHED_NC = _build_nc()
    return _CACHED_NC


def _fold2d(a):
    """[(t p), X] -> [p, t*X] partition-folded contiguous."""
    t = a.shape[0] // PT
    return np.ascontiguousarray(
        a.reshape(t, PT, a.shape[1]).transpose(1, 0, 2).reshape(
            PT, t * a.shape[1]))


def kernel(**inputs):
    global LAST_RESULT
    x = np.asarray(inputs["x"], np.float32)
    Wq = np.asarray(inputs["Wq"], np.float32)
    bq = np.asarray(inputs["bq"], np.float32)
    Wk = np.asarray(inputs["Wk"], np.float32)
    bk = np.asarray(inputs["bk"], np.float32)
    Wv = np.asarray(inputs["Wv"], np.float32)
    bv = np.asarray(inputs["bv"], np.float32)
    Wc = np.asarray(inputs["Wc"], np.float32)
    bc = np.asarray(inputs["bc"], np.float32)

    w1, w2 = Wc[0, :D], Wc[0, D:]
    # weights-only folding + O(N*D) vectors
    A = (Wq.T @ Wk).astype(np.float32)
    w = (x @ (Wk.T @ bq) + bq @ bk).astype(np.float32)   # [N]
    z = (x @ (Wv.T @ w1) + bv @ w1).astype(np.float32)   # [N]
    t2 = np.float64(w2 @ x.sum(axis=0, dtype=np.float64).astype(np.float32))

    x8 = (x * SX).astype(FP8_NP)
    a_h = _fold2d((A * SA).astype(FP8_NP)).astype(np.float32)  # [128, 2048]

    # xT stationaries: [p, rp, rsub, n] = x[n, (2rp+rsub)*128+p]
    xt_h = np.ascontiguousarray(
        x8.T.reshape(2, 2, PT, N).transpose(2, 0, 1, 3).reshape(PT, 4 * N))

    m2 = np.zeros((PT, PT + 3 * BP), np.float32)
    m2[:, 0:PT] = np.triu(np.ones((PT, PT), np.float32), 1)
    m2[0, 0] = 1.0   # keeps Lc_0 > 0 so 1/Lc is finite (mask kills j=0 anyway)
    jj = np.arange(N)
    sm = np.zeros((PT, 6), np.float32)
    for kc, ck in enumerate(CN):
        sm[0:ck, kc] = z[kc * PT:kc * PT + ck]
        sm[0:ck, 3 + kc] = SCALE * w[kc * PT:kc * PT + ck]

    in_maps = []
    for c in range(NCORES):
        i0 = c * B
        ig = i0 + np.arange(B)
        m2c = m2.copy()
        for jc, cn in enumerate(CN):
            jg = jc * PT + np.arange(cn)
            with np.errstate(divide="ignore"):
                m2c[0:cn, PT + jc * BP:PT + jc * BP + B] = np.where(
                    jg[:, None] > 0,
                    (ig[None, :] < jg[:, None]) / np.maximum(jg, 1)[:, None],
                    0.0)
        xtb_h = _fold2d(np.ascontiguousarray(
            np.pad(x8[i0:i0 + B].astype(np.float32),
                   ((0, BP - B), (0, 0))).T))
        m = {
            "ax": np.concatenate([a_h, xtb_h], axis=1).astype(FP8_NP),
            "xt": xt_h,
            "m2": m2c.astype(BF16_NP),
            "sm": sm,
        }
        in_maps.append(m)

    nc = _get_nc()
    trace = bool(int(os.environ.get("KERNEL_TRACE", "0")))
    trace_cores = None
    if trace:
        try:
            _ensure_ntff_hook()
        except Exception as e:
            print(f"ntff hook shim failed ({e!r}); running untraced")
            trace = False
        if int(os.environ.get("KERNEL_TRACE_ALL", "0")):
            trace_cores = list(range(NCORES))
    try:
        res = run_bass_kernel_spmd(
            nc, in_maps, core_ids=list(range(NCORES)),
            trace=trace, trace_cores=trace_cores,
        )
    except Exception as e:
        # Transient device errors (UNAVAILABLE / INTERNAL) occur on this
        # fabric; one retry on a fresh attempt is usually enough.
        print(f"run_bass_kernel_spmd failed ({type(e).__name__}); retrying once")
        res = run_bass_kernel_spmd(
            nc, in_maps, core_ids=list(range(NCORES)),
            trace=False, trace_cores=None,
        )
    LAST_RESULT = res
    total = np.float64(0.0)
    for c in range(NCORES):
        total += np.float64(res.results[c]["out"].sum(dtype=np.float64))
    total += t2 + np.float64(N) * np.float64(bc[0])
    return np.array([total], dtype=np.float32)
